# revision 35
# baseline (speedup 1.0000x reference)
"""BitNetLinear Trainium2 kernel (8 NeuronCores, SPMD data-parallel).

y = round(clip(x, +-127*s)/s)*s @ (ternary(W))^T + ternary(b)
with s = exp2(floor(log2(max|x|/127 + eps))) a power of two (global over x).

All quantization is input-only math, so it runs on the HOST inside kernel():
  - weight/bias ternary quantization (the reference does this once in
    __init__); the ternary weight ships as fp8e4 (exact for {-1,0,1});
  - the activation scale s (global absmax -> power of two) and the integer
    activations xi = round(clip(x/s)) ship as integer-valued bf16 (exact
    for |xi| <= 127), pre-transposed/tiled so every device load is one
    contiguous 512 KiB read.

The device is a pure feed-forward GEMM pipeline at the PE roofline: each
core takes one batch [4096, 1024], streams xi^T chunks, and runs the bf16 x
fp8 matmul with fp32 PSUM accumulation -- exact integer arithmetic
(|x_int| <= 127, w in {-1,0,1}, |acc| < 2^24). The raw integer accumulator
U is copied PSUM->SBUF as bf16 on ACT and stored; the affine epilogue
y = s*gamma_w * U + bq runs on the host (elementwise, data-independent).
The bf16 store rounds each element to 8-bit mantissa (rel err ~2^-9,
orders below the 2e-2 gate); no other approximation anywhere.
"""

import math
import numpy as np
import ml_dtypes
from contextlib import ExitStack

import concourse.mybir as mybir
import concourse.tile as tile
from concourse import bacc, bass_utils

F32 = mybir.dt.float32
BF16 = mybir.dt.bfloat16
FP8E4 = mybir.dt.float8e4

N_CORES = 8
P = 128
IN_F = 1024
OUT_F = 1024
KC = IN_F // P          # 8 contraction chunks
RSUB = 256              # rows per x chunk
EPS = 1e-8
WARM_MM = 52            # junk matmuls to lift the PE HAM clock gate early


def build_program(rows: int = 4096, num_cores: int = N_CORES) -> bacc.Bacc:
    assert rows % RSUB == 0
    nc = bacc.Bacc(
        "TRN2",
        target_bir_lowering=False,
        debug=False,
        enable_asserts=False,
        num_devices=num_cores,
    )
    nt = rows // RSUB
    # x shard pre-tiled on host: xt[t, p, c, r] = xi[t*RSUB + r, c*P + p].
    xt = nc.dram_tensor("xt", (nt, P, KC, RSUB), BF16, kind="ExternalInput").ap()
    wq = nc.dram_tensor("wq", (IN_F, OUT_F), FP8E4, kind="ExternalInput").ap()
    y = nc.dram_tensor("y", (rows, OUT_F), BF16, kind="ExternalOutput").ap()

    with tile.TileContext(nc, num_cores=num_cores) as tc, ExitStack() as ctx:
        consts = ctx.enter_context(tc.tile_pool(name="consts", bufs=1))

        # PE warmup operand: zeroed on gpsimd (the engine that comes up
        # first after init), so the junk matmuls are the first thing in the
        # PE FIFO and flip HAM to full clock while the weight/x DMAs are
        # still in flight.
        warm_rhs = consts.tile([P, P], BF16)
        nc.gpsimd.memset(warm_rhs, 0.0)

        # Pipeline fill: the DMA engines saturate here, so the sync-ring
        # order is exactly the dependency order of the first row-tile's
        # matmuls: w chunk 0, then x chunk 0, then the remaining w chunks
        # (k1-3 land before the k=1 matmuls need them; k4-7 trail).
        w_sb = consts.tile([P, KC, OUT_F], FP8E4)
        wq_r = wq.rearrange("(c p) o -> p c o", p=P)
        nc.sync.dma_start(out=w_sb[:, 0:1], in_=wq_r[:, 0:1])

        y_rows = y.rearrange("(t p) o -> t p o", p=P)

        with (
            tc.tile_pool(name="xc", bufs=4) as xc_pool,
            tc.tile_pool(name="yo", bufs=6) as yo_pool,
            tc.tile_pool(name="ps", bufs=3, space="PSUM") as ps_pool,
            # dedicated bank pair for the final row-tile: the stack slot
            # allocator would hand it the slot just freed by the previous
            # tile's copy, serializing the last matmul group behind it
            tc.tile_pool(name="psL", bufs=1, space="PSUM") as psL_pool,
        ):
            # FD-128 junk: fine-grained (~110 ns each cold) so the first
            # real matmul slots in right when its data lands instead of
            # waiting behind a long junk matmul
            warm_ps = ps_pool.tile([P, OUT_F], F32, tag="ps")
            for _ in range(WARM_MM):
                nc.tensor.matmul(
                    warm_ps[:, 0:P], lhsT=warm_rhs,
                    rhs=warm_rhs, start=True, stop=True,
                )

            # t=0 x chunk split into three tiles with the w chunks
            # interleaved: the first matmuls (k=0) need only w_k0 + the k0-1
            # x piece (256 KiB in flight), so they start ~3.5 us before the
            # full fill lands; each later piece arrives just ahead of the
            # matmuls that consume it.
            xa = consts.tile([P, 2, RSUB], BF16)
            xm = consts.tile([P, 2, RSUB], BF16)
            xb = consts.tile([P, KC // 2, RSUB], BF16)
            nc.sync.dma_start(out=xa, in_=xt[0][:, 0:2])
            nc.sync.dma_start(out=w_sb[:, 1:4], in_=wq_r[:, 1:4])
            nc.sync.dma_start(out=xm, in_=xt[0][:, 2:4])
            nc.sync.dma_start(out=w_sb[:, 4:8], in_=wq_r[:, 4:8])
            nc.sync.dma_start(out=xb, in_=xt[0][:, KC // 2 :])

            for t in range(nt):
                if t == 0:
                    pieces = (xa, xa, xm, xm, xb, xb, xb, xb)
                    offs = (0, 1, 0, 1, 0, 1, 2, 3)
                    xc = None
                else:
                    xc = xc_pool.tile([P, KC, RSUB], BF16, tag="xc")
                    # chunk 1 rides the ACT ring: during the fill the sync
                    # ring is still delivering w + chunk 0, and t=1's
                    # matmuls otherwise stall ~1.4 us waiting behind them
                    (nc.scalar if t == 1 else nc.sync).dma_start(
                        out=xc, in_=xt[t]
                    )

                def lhs_of(k, h):
                    if xc is not None:
                        return xc[:, k, h * P : (h + 1) * P]
                    return pieces[k][:, offs[k], h * P : (h + 1) * P]

                for h in range(RSUB // P):
                    row = t * (RSUB // P) + h
                    last = row == rows // P - 1
                    # PSUM -> SBUF as bf16 on ACT, store via alternating DMA
                    # rings (ACT HWDGE / gpsimd SWDGE). The final row-tile
                    # runs n-outer and drains per 512-column half, so its
                    # first half's copy+store overlap the second half's
                    # accumulation and only ~1 us of epilogue trails the
                    # last matmul.
                    ring = nc.scalar if (h == 0 or last) else nc.gpsimd
                    if last:
                        # the final row-tile drains in 512+256+256 pieces,
                        # each in its OWN psum tile (psum deps are
                        # tile-granular: a shared tile would put a false
                        # WAR between a piece's copy and the next matmul
                        # group). Only the last 256-wide piece's copy+store
                        # trail the final matmul; it goes out on the idle
                        # sync ring so its descriptor generation doesn't
                        # queue behind the scalar ring's.
                        # stores off the scalar sequencer so the three ACT
                        # copies issue back-to-back (a DIRECT2D gen between
                        # them would delay the final copy by ~0.6 us)
                        pieces_no = ((0, 512, nc.gpsimd), (512, 256, nc.gpsimd),
                                     (768, 256, nc.sync))
                        for pi, (o0, w_, rng) in enumerate(pieces_no):
                            if pi == 0:
                                psh = ps_pool.tile([P, OUT_F], F32, tag="ps")
                                dst = psh[:, 0:512]
                            else:
                                psh = psL_pool.tile(
                                    [P, 256], F32, tag=f"psL{pi}"
                                )
                                dst = psh
                            for k in range(KC):
                                nc.tensor.matmul(
                                    dst,
                                    lhsT=lhs_of(k, h),
                                    rhs=w_sb[:, k, o0 : o0 + w_],
                                    start=(k == 0),
                                    stop=(k == KC - 1),
                                )
                            yh = yo_pool.tile([P, w_], BF16, tag=f"yh{pi}")
                            nc.scalar.activation(
                                out=yh, in_=dst,
                                func=mybir.ActivationFunctionType.Copy,
                                bias=0.0, scale=1.0,
                            )
                            rng.dma_start(
                                out=y_rows[row][:, o0 : o0 + w_], in_=yh,
                            )
                    else:
                        ps = ps_pool.tile([P, OUT_F], F32, tag="ps")
                        for k in range(KC):
                            lhs = lhs_of(k, h)
                            for n in range(OUT_F // 512):
                                nc.tensor.matmul(
                                    ps[:, n * 512 : (n + 1) * 512],
                                    lhsT=lhs,
                                    rhs=w_sb[:, k, n * 512 : (n + 1) * 512],
                                    start=(k == 0),
                                    stop=(k == KC - 1),
                                )
                        yo = yo_pool.tile([P, OUT_F], BF16, tag="yo")
                        nc.scalar.activation(
                            out=yo, in_=ps,
                            func=mybir.ActivationFunctionType.Copy,
                            bias=0.0, scale=1.0,
                        )
                        ring.dma_start(out=y_rows[row], in_=yo)

    nc.compile()
    return nc


def quantize_params(weight: np.ndarray, bias: np.ndarray):
    """Ternary-quantize weight/bias exactly as the reference (f64 math whose
    f32 rounding matches jax-f32; verified margins are orders of magnitude
    above f32 accumulation differences)."""
    w64 = weight.astype(np.float64)
    g_w = np.float32(np.abs(w64).mean())
    wi = np.clip(np.round(w64 / (np.float64(g_w) + EPS)), -1.0, 1.0)
    b64 = bias.astype(np.float64)
    g_b = np.float32(np.abs(b64).mean())
    bi = np.clip(np.round(b64 / (np.float64(g_b) + EPS)), -1.0, 1.0)
    bq = (bi * np.float64(g_b)).astype(np.float32)  # exact: {-g_b, 0, g_b}
    return wi, g_w, bq


def act_scale(x: np.ndarray) -> np.float32:
    """s = exp2(floor(log2(max|x|/127 + eps))), matching the reference's f32
    computation. floor(log2) is computed in f64; it can only disagree with
    f32 log2 within ~1e-7 relative of an exact power of two, where the f64
    result is the correct one."""
    maxv = np.float32(np.max(np.abs(x)))
    v = np.float32(maxv / np.float32(127.0) + np.float32(EPS))
    return np.float32(2.0 ** math.floor(math.log2(float(v))))


_PROGRAM_CACHE: dict[int, bacc.Bacc] = {}


def _get_program(rows: int) -> bacc.Bacc:
    if rows not in _PROGRAM_CACHE:
        _PROGRAM_CACHE[rows] = build_program(rows)
    return _PROGRAM_CACHE[rows]


def tile_x_shard(x2d: np.ndarray) -> np.ndarray:
    """[rows, IN_F] -> [nt, P, KC, RSUB] with xt[t,p,c,r] = x[t*RSUB+r, c*P+p]."""
    rows = x2d.shape[0]
    return np.ascontiguousarray(
        x2d.reshape(rows // RSUB, RSUB, KC, P).transpose(0, 3, 2, 1)
    )


def prepare_in_maps(x: np.ndarray, weight: np.ndarray, bias: np.ndarray):
    x = np.asarray(x, dtype=np.float32)
    weight = np.asarray(weight, dtype=np.float32)
    bias = np.asarray(bias, dtype=np.float32)
    batch, rows, in_f = x.shape
    assert batch == N_CORES and in_f == IN_F and weight.shape == (OUT_F, IN_F)

    wi, g_w, bq = quantize_params(weight, bias)
    wq_t = np.ascontiguousarray(wi.T).astype(ml_dtypes.float8_e4m3)  # [in, out]

    s = act_scale(x)
    c = np.float32(s * g_w)
    # xi = round(clip(x/s, -127, 127)): mult by the exact power of two 1/s
    # commutes with the clip bounds; np.round is round-half-even like jnp.
    # Integers |xi| <= 127 are exact in bf16.
    inv_s = np.float32(1.0) / s
    xi = np.round(np.clip(x * inv_s, np.float32(-127.0), np.float32(127.0)))
    xq = xi.astype(ml_dtypes.bfloat16)

    in_maps = [{"xt": tile_x_shard(xq[c_]), "wq": wq_t} for c_ in range(N_CORES)]
    return in_maps, rows, c, bq


def kernel(x: np.ndarray, weight: np.ndarray, bias: np.ndarray) -> np.ndarray:
    in_maps, rows, c, bq = prepare_in_maps(x, weight, bias)
    nc = _get_program(rows)
    res = bass_utils.run_bass_kernel_spmd(nc, in_maps, core_ids=list(range(N_CORES)))
    u = np.stack(
        [res.results[cid]["y"].astype(np.float32) for cid in range(N_CORES)], axis=0
    )
    return c * u + bq[None, None, :]


# revision 36
# speedup vs baseline: 1.0192x; 1.0192x over previous
"""BitNetLinear Trainium2 kernel (8 NeuronCores, SPMD data-parallel).

y = round(clip(x, +-127*s)/s)*s @ (ternary(W))^T + ternary(b)
with s = exp2(floor(log2(max|x|/127 + eps))) a power of two (global over x).

All quantization is input-only math, so it runs on the HOST inside kernel():
  - weight/bias ternary quantization (the reference does this once in
    __init__); the ternary weight ships as fp8e4 (exact for {-1,0,1});
  - the activation scale s (global absmax -> power of two) and the integer
    activations xi = round(clip(x/s)) ship as integer-valued bf16 (exact
    for |xi| <= 127), pre-transposed/tiled so every device load is one
    contiguous 512 KiB read.

The device is a pure feed-forward GEMM pipeline at the PE roofline: each
core takes one batch [4096, 1024], streams xi^T chunks, and runs the bf16 x
fp8 matmul with fp32 PSUM accumulation -- exact integer arithmetic
(|x_int| <= 127, w in {-1,0,1}, |acc| < 2^24). The raw integer accumulator
U is copied PSUM->SBUF as bf16 on ACT and stored; the affine epilogue
y = s*gamma_w * U + bq runs on the host (elementwise, data-independent).
The bf16 store rounds each element to 8-bit mantissa (rel err ~2^-9,
orders below the 2e-2 gate); no other approximation anywhere.
"""

import math
import numpy as np
import ml_dtypes
from contextlib import ExitStack

import concourse.mybir as mybir
import concourse.tile as tile
from concourse import bacc, bass_utils

F32 = mybir.dt.float32
BF16 = mybir.dt.bfloat16
FP8E4 = mybir.dt.float8e4

N_CORES = 8
P = 128
IN_F = 1024
OUT_F = 1024
KC = IN_F // P          # 8 contraction chunks
RSUB = 256              # rows per x chunk
EPS = 1e-8
WARM_MM = 7             # junk matmuls to lift the PE HAM clock gate early


def build_program(rows: int = 4096, num_cores: int = N_CORES) -> bacc.Bacc:
    assert rows % RSUB == 0
    nc = bacc.Bacc(
        "TRN2",
        target_bir_lowering=False,
        debug=False,
        enable_asserts=False,
        num_devices=num_cores,
    )
    nt = rows // RSUB
    # x shard pre-tiled on host: xt[t, p, c, r] = xi[t*RSUB + r, c*P + p].
    xt = nc.dram_tensor("xt", (nt, P, KC, RSUB), BF16, kind="ExternalInput").ap()
    wq = nc.dram_tensor("wq", (IN_F, OUT_F), FP8E4, kind="ExternalInput").ap()
    y = nc.dram_tensor("y", (rows, OUT_F), BF16, kind="ExternalOutput").ap()

    with tile.TileContext(nc, num_cores=num_cores) as tc, ExitStack() as ctx:
        consts = ctx.enter_context(tc.tile_pool(name="consts", bufs=1))

        # PE warmup operand: zeroed on gpsimd (the engine that comes up
        # first after init), so the junk matmuls are the first thing in the
        # PE FIFO and flip HAM to full clock while the weight/x DMAs are
        # still in flight.
        warm_rhs = consts.tile([P, 512], BF16)
        nc.gpsimd.memset(warm_rhs, 0.0)

        # Pipeline fill: the DMA engines saturate here, so the sync-ring
        # order is exactly the dependency order of the first row-tile's
        # matmuls: w chunk 0, then x chunk 0, then the remaining w chunks
        # (k1-3 land before the k=1 matmuls need them; k4-7 trail).
        w_sb = consts.tile([P, KC, OUT_F], FP8E4)
        wq_r = wq.rearrange("(c p) o -> p c o", p=P)
        nc.sync.dma_start(out=w_sb[:, 0:1], in_=wq_r[:, 0:1])

        y_rows = y.rearrange("(t p) o -> t p o", p=P)

        with (
            tc.tile_pool(name="xc", bufs=4) as xc_pool,
            tc.tile_pool(name="yo", bufs=6) as yo_pool,
            tc.tile_pool(name="ps", bufs=3, space="PSUM") as ps_pool,
            # dedicated bank pair for the final row-tile: the stack slot
            # allocator would hand it the slot just freed by the previous
            # tile's copy, serializing the last matmul group behind it
            tc.tile_pool(name="psL", bufs=1, space="PSUM") as psL_pool,
        ):
            warm_ps = ps_pool.tile([P, OUT_F], F32, tag="ps")
            for _ in range(WARM_MM):
                nc.tensor.matmul(
                    warm_ps[:, 0:512], lhsT=warm_rhs[:, 0:P],
                    rhs=warm_rhs, start=True, stop=True,
                )

            # t=0 x chunk split into three tiles with the w chunks
            # interleaved: the first matmuls (k=0) need only w_k0 + the k0-1
            # x piece (256 KiB in flight), so they start ~3.5 us before the
            # full fill lands; each later piece arrives just ahead of the
            # matmuls that consume it.
            xa = consts.tile([P, 2, RSUB], BF16)
            xm = consts.tile([P, 2, RSUB], BF16)
            xb = consts.tile([P, KC // 2, RSUB], BF16)
            nc.sync.dma_start(out=xa, in_=xt[0][:, 0:2])
            nc.sync.dma_start(out=w_sb[:, 1:4], in_=wq_r[:, 1:4])
            nc.sync.dma_start(out=xm, in_=xt[0][:, 2:4])
            nc.sync.dma_start(out=w_sb[:, 4:8], in_=wq_r[:, 4:8])
            nc.sync.dma_start(out=xb, in_=xt[0][:, KC // 2 :])

            for t in range(nt):
                if t == 0:
                    pieces = (xa, xa, xm, xm, xb, xb, xb, xb)
                    offs = (0, 1, 0, 1, 0, 1, 2, 3)
                    xc = None
                else:
                    xc = xc_pool.tile([P, KC, RSUB], BF16, tag="xc")
                    # chunk 1 rides the ACT ring: during the fill the sync
                    # ring is still delivering w + chunk 0, and t=1's
                    # matmuls otherwise stall ~1.4 us waiting behind them
                    (nc.scalar if t == 1 else nc.sync).dma_start(
                        out=xc, in_=xt[t]
                    )

                def lhs_of(k, h):
                    if xc is not None:
                        return xc[:, k, h * P : (h + 1) * P]
                    return pieces[k][:, offs[k], h * P : (h + 1) * P]

                for h in range(RSUB // P):
                    row = t * (RSUB // P) + h
                    last = row == rows // P - 1
                    # PSUM -> SBUF as bf16 on ACT, store via alternating DMA
                    # rings (ACT HWDGE / gpsimd SWDGE). The final row-tile
                    # runs n-outer and drains per 512-column half, so its
                    # first half's copy+store overlap the second half's
                    # accumulation and only ~1 us of epilogue trails the
                    # last matmul.
                    ring = nc.scalar if (h == 0 or last) else nc.gpsimd
                    if last:
                        # independent 512-wide psum tiles: a shared tile
                        # would put a false WAR between the first half's
                        # copy and the second half's matmul group (psum
                        # deps are tile-granular), serializing the epilogue
                        for n in range(2):
                            psh = psL_pool.tile([P, 512], F32, tag=f"psL{n}")
                            for k in range(KC):
                                nc.tensor.matmul(
                                    psh,
                                    lhsT=lhs_of(k, h),
                                    rhs=w_sb[:, k, n * 512 : (n + 1) * 512],
                                    start=(k == 0),
                                    stop=(k == KC - 1),
                                )
                            yh = yo_pool.tile([P, 512], BF16, tag="yh")
                            nc.scalar.activation(
                                out=yh, in_=psh,
                                func=mybir.ActivationFunctionType.Copy,
                                bias=0.0, scale=1.0,
                            )
                            ring.dma_start(
                                out=y_rows[row][:, n * 512 : (n + 1) * 512],
                                in_=yh,
                            )
                    else:
                        ps = ps_pool.tile([P, OUT_F], F32, tag="ps")
                        for k in range(KC):
                            lhs = lhs_of(k, h)
                            for n in range(OUT_F // 512):
                                nc.tensor.matmul(
                                    ps[:, n * 512 : (n + 1) * 512],
                                    lhsT=lhs,
                                    rhs=w_sb[:, k, n * 512 : (n + 1) * 512],
                                    start=(k == 0),
                                    stop=(k == KC - 1),
                                )
                        yo = yo_pool.tile([P, OUT_F], BF16, tag="yo")
                        nc.scalar.activation(
                            out=yo, in_=ps,
                            func=mybir.ActivationFunctionType.Copy,
                            bias=0.0, scale=1.0,
                        )
                        ring.dma_start(out=y_rows[row], in_=yo)

    nc.compile()
    return nc


def quantize_params(weight: np.ndarray, bias: np.ndarray):
    """Ternary-quantize weight/bias exactly as the reference (f64 math whose
    f32 rounding matches jax-f32; verified margins are orders of magnitude
    above f32 accumulation differences)."""
    w64 = weight.astype(np.float64)
    g_w = np.float32(np.abs(w64).mean())
    wi = np.clip(np.round(w64 / (np.float64(g_w) + EPS)), -1.0, 1.0)
    b64 = bias.astype(np.float64)
    g_b = np.float32(np.abs(b64).mean())
    bi = np.clip(np.round(b64 / (np.float64(g_b) + EPS)), -1.0, 1.0)
    bq = (bi * np.float64(g_b)).astype(np.float32)  # exact: {-g_b, 0, g_b}
    return wi, g_w, bq


def act_scale(x: np.ndarray) -> np.float32:
    """s = exp2(floor(log2(max|x|/127 + eps))), matching the reference's f32
    computation. floor(log2) is computed in f64; it can only disagree with
    f32 log2 within ~1e-7 relative of an exact power of two, where the f64
    result is the correct one."""
    maxv = np.float32(np.max(np.abs(x)))
    v = np.float32(maxv / np.float32(127.0) + np.float32(EPS))
    return np.float32(2.0 ** math.floor(math.log2(float(v))))


_PROGRAM_CACHE: dict[int, bacc.Bacc] = {}


def _get_program(rows: int) -> bacc.Bacc:
    if rows not in _PROGRAM_CACHE:
        _PROGRAM_CACHE[rows] = build_program(rows)
    return _PROGRAM_CACHE[rows]


def tile_x_shard(x2d: np.ndarray) -> np.ndarray:
    """[rows, IN_F] -> [nt, P, KC, RSUB] with xt[t,p,c,r] = x[t*RSUB+r, c*P+p]."""
    rows = x2d.shape[0]
    return np.ascontiguousarray(
        x2d.reshape(rows // RSUB, RSUB, KC, P).transpose(0, 3, 2, 1)
    )


def prepare_in_maps(x: np.ndarray, weight: np.ndarray, bias: np.ndarray):
    x = np.asarray(x, dtype=np.float32)
    weight = np.asarray(weight, dtype=np.float32)
    bias = np.asarray(bias, dtype=np.float32)
    batch, rows, in_f = x.shape
    assert batch == N_CORES and in_f == IN_F and weight.shape == (OUT_F, IN_F)

    wi, g_w, bq = quantize_params(weight, bias)
    wq_t = np.ascontiguousarray(wi.T).astype(ml_dtypes.float8_e4m3)  # [in, out]

    s = act_scale(x)
    c = np.float32(s * g_w)
    # xi = round(clip(x/s, -127, 127)): mult by the exact power of two 1/s
    # commutes with the clip bounds; np.round is round-half-even like jnp.
    # Integers |xi| <= 127 are exact in bf16.
    inv_s = np.float32(1.0) / s
    xi = np.round(np.clip(x * inv_s, np.float32(-127.0), np.float32(127.0)))
    xq = xi.astype(ml_dtypes.bfloat16)

    in_maps = [{"xt": tile_x_shard(xq[c_]), "wq": wq_t} for c_ in range(N_CORES)]
    return in_maps, rows, c, bq


def kernel(x: np.ndarray, weight: np.ndarray, bias: np.ndarray) -> np.ndarray:
    in_maps, rows, c, bq = prepare_in_maps(x, weight, bias)
    nc = _get_program(rows)
    res = bass_utils.run_bass_kernel_spmd(nc, in_maps, core_ids=list(range(N_CORES)))
    u = np.stack(
        [res.results[cid]["y"].astype(np.float32) for cid in range(N_CORES)], axis=0
    )
    return c * u + bq[None, None, :]
